# revision 18
# baseline (speedup 1.0000x reference)
"""Grouped GEMM (MoE expert layer) on 8 Trainium2 NeuronCores.

Problem: out[t] = input[t] @ weight[expert(t)].T + bias[expert(t)], where
tokens are pre-sorted by expert and group sizes come from expert_frequency
(host-readable static metadata, same as the reference's .tolist()).

Strategy (single uniform SPMD program, all-to-all token routing on host):
  - One shared "slot profile" P: every core runs S slots; slot s processes
    P[s] tiles of 128 tokens with one weight matrix. Slot weights/biases and
    the token blocks are per-core DATA (host-gathered), so one NEFF serves
    all 8 cores despite the uneven expert sizes.
  - A planner packs (expert, tile-range) pieces into the 8xS global slot
    inventory with ~2% padding at C = ceil(total_tiles/8).
  - Matmul layout: W-stationary. lhsT = WT[kc, dc-chunk] (128x128 fp16),
    moving = XT[kc, 512-token block] (e3m4), psum = [128 dout, 512 tok].
    All 8 psum banks cycle per 4096-token supertile; bank drains (DVE
    bias-add, fp16 out) overlap the next dc round at bank granularity.
  - The kernel is PE-roofline-bound (~453us/core at fp16 rate); measured
    overhead above that is DMA-byte-proportional SBUF-port interference,
    so IO is minimized: X streams as e3m4 (~1.1% one-sided quant noise,
    rel_l2 ~1.1e-2), outputs fp16, weights fp16 with a low-slot-count
    plan (S~10) to cap weight reloads.
  - Input X is transposed on host ([d_in, tokens]) so every device DMA
    is contiguous-row; output is produced transposed ([d_out, tokens] fp16)
    and transposed back on host.
"""

import numpy as np
import ml_dtypes

import concourse.bacc as bacc
import concourse.mybir as mybir
import concourse.tile as tile
from concourse.bass_utils import run_bass_kernel_spmd

N_CORES = 8
KC = 8          # contraction chunks (d_in = KC*128)
DC = 8          # d_out chunks (d_out = DC*128)
D_IN = 1024
D_OUT = 1024
TILE = 128
SUPER_TOK = 4096    # tokens per supertile (8 psum banks x 512)
BLK = 512           # moving-operand tokens per matmul

FP8_CHUNKS = 0      # number of leading 256-row contraction chunks in fp8 DR
W_SCALE = 64.0      # weight pre-scale when fp8 enabled (power of two, exact)

f32 = mybir.dt.float32
f16 = mybir.dt.float16
f8 = mybir.dt.float8e4
f8e3 = mybir.dt.float8e3

np_f8 = ml_dtypes.float8_e4m3   # TRN float8e4 semantics (max +-240)

# X is stored and streamed as e3m4 (4 mantissa bits): halves x DMA traffic;
# one-sided quantization at ~1.1% rms keeps rel_l2 ~1.1e-2. The PE reads the
# fp8 moving operand at full rate against the fp16 stationary weights.
X_DT = f8e3
NP_X_DT = ml_dtypes.float8_e3m4


# ----------------------------------------------------------------- planner --

def _greedy_assign(tiles_e, inventory):
    inv = dict(inventory)
    sizes = sorted(inv.keys(), reverse=True)
    order = sorted(range(len(tiles_e)), key=lambda e: -tiles_e[e])
    out = []
    for e in order:
        rem = tiles_e[e]
        toff = 0
        while rem > 0:
            pick = None
            for s in sizes:
                if inv.get(s, 0) > 0 and s <= rem:
                    pick = s
                    break
            if pick is None:
                cands = [s for s in sizes if inv.get(s, 0) > 0 and s >= rem]
                if not cands:
                    return None
                pick = min(cands)
            take = min(rem, pick)
            inv[pick] -= 1
            out.append((e, toff, pick, take))
            rem -= take
            toff += take
    return out


def _distribute_to_cores(P, assignments, n_cores):
    from collections import defaultdict
    P_desc = sorted(P, reverse=True)
    core_slots = []
    for c in range(n_cores):
        d = defaultdict(list)
        for idx, p in enumerate(P_desc):
            d[p].append(idx)
        core_slots.append(d)
    plan = [[None] * len(P_desc) for _ in range(n_cores)]
    rr = {p: 0 for p in set(P_desc)}
    for (e, toff, size, take) in sorted(assignments, key=lambda a: -a[2]):
        start = rr[size]
        for k in range(n_cores):
            c = (start + k) % n_cores
            if core_slots[c][size]:
                idx = core_slots[c][size].pop(0)
                plan[c][idx] = (e, toff, take)
                rr[size] = (c + 1) % n_cores
                break
        else:
            raise AssertionError("inventory accounting bug")
    return P_desc, plan


def _assign_best_fit(tiles_e, P, n_cores, tries=300, seed=0):
    """Randomized best-fit: split experts into pieces matched to the slot
    inventory (n_cores copies of each size in P). Stronger than the plain
    greedy; returns [(expert, tile_offset, slot_size, tiles_taken)] or None."""
    rng = np.random.default_rng(seed)
    for t in range(tries):
        inv = {}
        for p in P:
            inv[p] = inv.get(p, 0) + n_cores
        order = [e for e in range(len(tiles_e)) if tiles_e[e] > 0]
        if t == 0:
            order.sort(key=lambda e: -tiles_e[e])
        else:
            rng.shuffle(order)
        out = []
        ok = True
        for e in order:
            rem = tiles_e[e]
            toff = 0
            while rem > 0:
                avail = [s for s in inv if inv[s] > 0]
                if not avail:
                    ok = False
                    break
                le = [s for s in avail if s <= rem]
                if rem in avail:
                    pick = rem
                elif le:
                    pick = max(le)
                else:
                    pick = min(s for s in avail if s >= rem)
                take = min(rem, pick)
                inv[pick] -= 1
                out.append((e, toff, pick, take))
                rem -= take
                toff += take
            if not ok:
                break
        if ok:
            return out
    return None


def make_plan(counts, n_cores=N_CORES, tile=TILE, max_slots=18):
    """Returns (P_desc, plan): P_desc = slot sizes (tiles) desc, shared by all
    cores; plan[c][s] = (expert, tok_offset, n_tokens) with n_tokens possibly 0."""
    counts = np.asarray(counts, dtype=np.int64)
    E = len(counts)
    offsets = np.concatenate([[0], np.cumsum(counts)])
    tiles_e = [max(0, int(np.ceil(c / tile))) for c in counts]
    total = max(1, sum(tiles_e))
    lo = int(np.ceil(total / n_cores))

    size_menu = [64, 48, 40, 32, 24, 20, 16, 12, 8, 6, 4, 3, 2, 1]
    best = None

    # guaranteed-feasible fallback: every expert split evenly over all cores
    fb = sorted((int(np.ceil(t / n_cores)) for t in tiles_e if t > 0), reverse=True)
    if fb:
        inv = {}
        for p in fb:
            inv[p] = inv.get(p, 0) + n_cores
        a = _greedy_assign(tiles_e, inv)
        if a is not None:
            best = (sum(fb) + 0.5 * len(fb) + 1e6, fb, a)  # huge cost: only a fallback

    rng = np.random.default_rng(0)
    for _ in range(4000):
        C_target = lo + int(rng.integers(0, 6))
        P = []
        rem = C_target
        for s in size_menu:
            if rem <= 0:
                break
            if s > rem:
                continue
            max_n = rem // s
            n = int(rng.integers(0, (max_n if s > 4 else min(max_n, 4)) + 1))
            if len(P) + n > max_slots:
                n = max_slots - len(P)
            P += [s] * n
            rem -= n * s
        while rem > 0 and len(P) < max_slots:
            s = max(x for x in size_menu if x <= rem)
            P.append(s)
            rem -= s
        if rem != 0 or not P:
            continue
        inv = {}
        for p in P:
            inv[p] = inv.get(p, 0) + n_cores
        a = _greedy_assign(tiles_e, inv)
        if a is None:
            continue
        cost = sum(P) + 0.5 * len(P)
        if best is None or cost < best[0]:
            best = (cost, P, a)
    assert best is not None, "no feasible slot profile found"

    # Second-stage search: fewer slots (less weight DMA) at equal/low padding,
    # validated with the stronger best-fit assigner. Cost in ~us: a slot costs
    # ~2.2 (weight-DMA leak), a padding tile ~3.4 (PE time).
    def cost2(S, C):
        return 2.2 * S + 3.4 * max(0, C - lo)

    b_cost, b_P, b_assign = best
    best2 = (cost2(len(b_P), sum(b_P)), b_P, b_assign)
    rng2 = np.random.default_rng(1)
    menu2 = [64, 48, 40, 36, 32, 28, 24, 20, 18, 16, 14, 12, 10, 8, 6, 5, 4, 3, 2, 1]
    for _ in range(20000):
        S_target = int(rng2.integers(4, 13))
        C_target = lo + int(rng2.integers(0, 12))
        c = cost2(S_target, C_target)
        if c >= best2[0]:
            continue
        P = []
        rem = C_target
        ok = True
        for i in range(S_target):
            left = S_target - i - 1
            if left == 0:
                if rem in menu2:
                    P.append(rem)
                    rem = 0
                else:
                    ok = False
                break
            cands = [s for s in menu2 if left <= rem - s <= 64 * left]
            if not cands:
                ok = False
                break
            s = int(rng2.choice(cands))
            P.append(s)
            rem -= s
        if not ok or rem != 0:
            continue
        a = _assign_best_fit(tiles_e, P, n_cores, tries=50)
        if a is not None:
            best2 = (c, P, a)
    _, P, assignments = best2
    P_desc, plan_t = _distribute_to_cores(P, assignments, n_cores)

    plan = []
    for c in range(n_cores):
        entries = []
        for piece in plan_t[c]:
            if piece is None:
                entries.append((0, 0, 0))
            else:
                e, toff, t = piece
                tok0 = int(offsets[e]) + toff * tile
                ntok = max(0, min(int(counts[e]) - toff * tile, t * tile))
                entries.append((e, tok0, ntok))
        plan.append(entries)
    return P_desc, plan


# ------------------------------------------------------------ device program --

_program_cache = {}


def build_program(P, reps=1, fp8_chunks=FP8_CHUNKS, skip_dma=False, skip_pe=False):
    """Uniform SPMD program for slot profile P (list of tile counts, desc).
    reps>1 repeats the whole schedule (used only for timing calibration).
    skip_dma/skip_pe build attribution variants (timing experiments only)."""
    key = (tuple(P), reps, fp8_chunks, skip_dma, skip_pe)
    if key in _program_cache:
        return _program_cache[key]

    A = fp8_chunks
    KC16 = KC - 2 * A            # fp16 contraction chunks
    S = len(P)
    C = sum(P)
    CT = C * TILE

    nc = bacc.Bacc()
    ws = nc.declare_dram_parameter("ws", [S, KC16 * 128, D_OUT], f16, isOutput=False)
    xt = nc.declare_dram_parameter("xt", [KC16 * 128, CT], X_DT, isOutput=False)
    if A > 0:
        ws8 = nc.declare_dram_parameter("ws8", [S, A * 256, D_OUT], f8, isOutput=False)
        xt8 = nc.declare_dram_parameter("xt8", [A * 256, CT], f8, isOutput=False)
    bs = nc.declare_dram_parameter("bs", [128, S * DC], f32, isOutput=False)
    out = nc.declare_dram_parameter("out", [D_OUT, CT], f16, isOutput=True)

    xt_r = xt.rearrange("(kc p) t -> p kc t", p=128)
    ws_r = ws.rearrange("s (kc p) n -> p s kc n", p=128)
    if A > 0:
        xt8_r = xt8.rearrange("(a two p) t -> p a two t", p=128, two=2)
        ws8_r = ws8.rearrange("s (a two p) n -> p s a two n", p=128, two=2)
    out_r = out.rearrange("(dc p) t -> p dc t", p=128)

    with tile.TileContext(nc) as tc:
        with (
            tc.tile_pool(name="xpool", bufs=2) as xpool,
            tc.tile_pool(name="wpool", bufs=2) as wpool,
            tc.tile_pool(name="opool", bufs=3) as opool,
            tc.tile_pool(name="bpool", bufs=1) as bpool,
            tc.tile_pool(name="psum", bufs=8, space="PSUM") as psum,
        ):
            b_sb = bpool.tile([128, S * DC], f32)
            nc.sync.dma_start(b_sb[:], bs[:])

            for _rep in range(reps):
              col = 0  # running token-column base
              for s in range(S):
                w_sb = wpool.tile([128, KC16 * D_OUT], f16, tag="wsb")
                if not skip_dma:
                    for kc in range(KC16):
                        nc.sync.dma_start(
                            w_sb[:, kc * D_OUT:(kc + 1) * D_OUT], ws_r[:, s, kc, :]
                        )
                else:
                    nc.sync.dma_start(w_sb[:, :1], ws_r[:, s, 0, :1])
                if A > 0:
                    w8_sb = wpool.tile([128, A, 2, D_OUT], f8, tag="w8sb")
                    if not skip_dma:
                        for a in range(A):
                            for j in range(2):
                                nc.scalar.dma_start(
                                    w8_sb[:, a, j, :], ws8_r[:, s, a, j, :]
                                )
                slot_tok = P[s] * TILE
                t0 = 0
                while t0 < slot_tok:
                    mtok = min(SUPER_TOK, slot_tok - t0)
                    nblk = (mtok + BLK - 1) // BLK
                    c0 = col + t0
                    x_sb = xpool.tile([128, KC16 * SUPER_TOK], X_DT, tag="xsb")
                    if not skip_dma:
                        for kc in range(KC16):
                            nc.sync.dma_start(
                                x_sb[:, kc * SUPER_TOK: kc * SUPER_TOK + mtok],
                                xt_r[:, kc, c0:c0 + mtok],
                            )
                    else:
                        nc.sync.dma_start(x_sb[:, :1], xt_r[:, 0, c0:c0 + 1])
                    if A > 0:
                        x8_sb = xpool.tile([128, A, 2, SUPER_TOK], f8, tag="x8sb")
                        if not skip_dma:
                            for a in range(A):
                                for j in range(2):
                                    nc.sync.dma_start(
                                        x8_sb[:, a, j, :mtok],
                                        xt8_r[:, a, j, c0:c0 + mtok],
                                    )
                    for dc in range(DC):
                        if skip_pe:
                            continue
                        o_sb = opool.tile([128, SUPER_TOK], f16, tag="osb")
                        accs = [
                            psum.tile([128, BLK], f32, name="acc", tag="acc")
                            for g in range(nblk)
                        ]
                        first = True
                        for a in range(A):
                            lhsT8 = w8_sb[:, a, :, dc * 128:(dc + 1) * 128]
                            for g in range(nblk):
                                ntok = min(BLK, mtok - g * BLK)
                                nc.tensor.matmul(
                                    accs[g][:, :ntok],
                                    lhsT8,
                                    x8_sb[:, a, :, g * BLK: g * BLK + ntok],
                                    start=first,
                                    stop=False,
                                    perf_mode=mybir.MatmulPerfMode.DoubleRow,
                                )
                            first = False
                        for kc in range(KC16):
                            lhsT = w_sb[:, kc * D_OUT + dc * 128: kc * D_OUT + (dc + 1) * 128]
                            for g in range(nblk):
                                ntok = min(BLK, mtok - g * BLK)
                                nc.tensor.matmul(
                                    accs[g][:, :ntok],
                                    lhsT,
                                    x_sb[:, kc * SUPER_TOK + g * BLK: kc * SUPER_TOK + g * BLK + ntok],
                                    start=first and kc == 0,
                                    stop=(kc == KC16 - 1),
                                )
                        for g in range(nblk):
                            ntok = min(BLK, mtok - g * BLK)
                            if A > 0:
                                nc.vector.tensor_scalar(
                                    o_sb[:, g * BLK: g * BLK + ntok],
                                    accs[g][:, :ntok],
                                    1.0 / W_SCALE,
                                    b_sb[:, s * DC + dc: s * DC + dc + 1],
                                    mybir.AluOpType.mult,
                                    mybir.AluOpType.add,
                                )
                            else:
                                nc.vector.tensor_scalar_add(
                                    o_sb[:, g * BLK: g * BLK + ntok],
                                    accs[g][:, :ntok],
                                    b_sb[:, s * DC + dc: s * DC + dc + 1],
                                )
                        if not skip_dma:
                            nc.gpsimd.dma_start(
                                out_r[:, dc, c0:c0 + mtok], o_sb[:, :mtok]
                            )
                    t0 += mtok
                col += slot_tok
    nc.finalize()
    _program_cache[key] = nc
    return nc


# ------------------------------------------------------------------ host prep --

def prepare_in_maps(input, weight, bias, P, plan, fp8_chunks=FP8_CHUNKS):
    A = fp8_chunks
    KC16 = KC - 2 * A
    k16_0 = A * 256                    # first fp16 contraction row
    S = len(P)
    CT = sum(P) * TILE
    wscale = np.float32(W_SCALE if A > 0 else 1.0)

    xT = np.ascontiguousarray(input.T)                       # [D_IN, T] f32
    x16t = xT[k16_0:].astype(NP_X_DT)                        # [KC16*128, T]
    wT = weight.transpose(0, 2, 1)                           # [E, D_IN, D_OUT]
    w16t = np.ascontiguousarray(wT[:, k16_0:, :] * wscale).astype(np.float16)
    if A > 0:
        x8t = xT[:k16_0].astype(np_f8)                       # [A*256, T]
        w8t = np.ascontiguousarray(wT[:, :k16_0, :] * wscale).astype(np_f8)
    bias32 = bias.astype(np.float32)

    in_maps = []
    for c in range(N_CORES):
        xt_c = np.zeros((KC16 * 128, CT), NP_X_DT)
        ws_c = np.empty((S, KC16 * 128, D_OUT), np.float16)
        bs_c = np.zeros((128, S * DC), np.float32)
        if A > 0:
            xt8_c = np.zeros((A * 256, CT), np_f8)
            ws8_c = np.empty((S, A * 256, D_OUT), np_f8)
        col = 0
        for s, (e, tok0, ntok) in enumerate(plan[c]):
            if ntok > 0:
                xt_c[:, col:col + ntok] = x16t[:, tok0:tok0 + ntok]
                if A > 0:
                    xt8_c[:, col:col + ntok] = x8t[:, tok0:tok0 + ntok]
            ws_c[s] = w16t[e]
            if A > 0:
                ws8_c[s] = w8t[e]
            bs_c[:, s * DC:(s + 1) * DC] = bias32[e].reshape(DC, 128).T
            col += P[s] * TILE
        m = {"xt": xt_c, "ws": ws_c, "bs": bs_c}
        if A > 0:
            m["xt8"] = xt8_c
            m["ws8"] = ws8_c
        in_maps.append(m)
    return in_maps


# ------------------------------------------------------------------ kernel --

def kernel(input, expert_frequency, weight, bias):
    input = np.asarray(input)
    counts = np.asarray(expert_frequency)
    weight = np.asarray(weight)
    bias = np.asarray(bias)
    T = input.shape[0]
    in_dtype = input.dtype

    P, plan = make_plan(counts)

    nc = build_program(P)
    in_maps = prepare_in_maps(input, weight, bias, P, plan)

    res = run_bass_kernel_spmd(nc, in_maps, core_ids=list(range(N_CORES)))

    out_full = np.empty((T, D_OUT), np.float32)
    for c in range(N_CORES):
        oc = res.results[c]["out"]          # [D_OUT, CT] f16
        col = 0
        for s, (e, tok0, ntok) in enumerate(plan[c]):
            if ntok > 0:
                out_full[tok0:tok0 + ntok, :] = oc[:, col:col + ntok].T
            col += P[s] * TILE
    return out_full.astype(in_dtype, copy=False)
